# revision 48
# baseline (speedup 1.0000x reference)
"""Trainium2 Bass kernel for nn_BerTII (masked-mean embedding bag -> 1-dim
linear -> sigmoid), distributed over 8 NeuronCores.

reference math:
  mask[b,l] = l < lengths[b]
  pooled[b,:] = sum_l mask[b,l] * emb[tokens[b,l],:] / L
  out[b] = sigmoid(pooled[b,:] @ W.T + bias)

v2 design (BERT_V=2, default; measured ~76us vs the 122us bf16 baseline):
  - host-side integer index marshaling: flatten valid (b,l) tokens, global
    np.unique dedupe (~60K unique rows of 200K vocab), multiplicity matrix
    cnt[U, B]; vocab-row-shard by DESCRIPTOR count into 8 contiguous chunks
    (the embedding table is staged per-core as only its vocab slice).
  - the gather's real cost on TRN2 is the Pool/Q7 SWDGE descriptor generation
    (~8.4ns/descriptor measured, independent of dtype/bytes); DMA bytes hide
    under it.  So the per-core slice (~30% row density) is gathered via
    MULTI-GAP SHINGLE TABLES: the host stages embg_k[v] = (slice[v],
    slice[v+k]) for k=1..3 (pure sliding-window views, fp8e4m3, rows padded
    1000->1024B, 2048B elements).  A greedy cover pairs each needed row with
    its successor whenever the gap is <=3, choosing the matching table;
    leftovers read gap-1 with a zero-count junk partner.  60358 rows ->
    36317 descriptors (~4.6K/core, 0.60/row vs 1.0 naive), emitted as one
    dma_gather call stream per gap class (small classes first, class 1 with
    a small-call tail last).
  - all reduction compute runs on the Tensor engine as fp8 DoubleRow matmuls
    (reduction-tile-2): pooled[B,1024] accumulates in two PSUM tile pairs via
    lhsT=cnt-slot [128,2,B] fp8, rhs=gathered pair [128,2,512] fp8; W is
    applied by scalar_tensor_tensor accum passes reading PSUM directly.
    The accumulation is phase-split so all but the last gather call's W-pass
    runs while the tail still gathers.
  - latency trims: the Q7 ucode library reload is hoisted into the preamble
    block post-compile (the ~9-12us Q7 boot overlaps input loads); the first
    gather call is small (first data lands ~7us sooner for the saturated PE);
    the first call's indices load as a separate tiny DMA; the [B] partial is
    PE-transposed to [1,B] so the output store is ONE 256B descriptor (a
    partition-major store's 64 tiny packets cost ~5us of completion lag).
  - each core emits its partial y[B] = pooled_c @ W; the host unshards by
    summing the 8 partials and applying sigmoid(x/L + b) (the output is
    sum-sharded across cores; no device collective - a warmed 256B ncfw
    AllGather measured 30+us on HW).
  - rejected experiments (measured): HWDGE streaming a slice block consumed
    by DVE (SBUF contention slows Q7 descgen ~28%; net loss), 2-bank [B,1024]
    PSUM matmul out (walrus verifier rejects), indirect_dma_start (same
    per-row Q7 cost), device AllGather/AllReduce finishes (tail >30us).

BERT_V=1 selects the previous sequence-sharded bf16 kernel (see bottom).
"""
import os
import sys

sys.path.insert(0, "/opt/trn_rl_repo")

import numpy as np

VOCAB = 200000
PDIM = 1000
PDIMP = 1024  # row padded to 1024 (one 256B-aligned fp8 gather unit)
PAIRE = 2048  # shingled pair element: 2 rows
B = 64
L = 2048
NCORES = 8
HALF = 512

LAST = {}  # debug: last BassKernelResults etc.


# ---------------------------------------------------------------------------
# walrus legalization: this toolchain allows at most ONE semaphore wait per
# instruction ("Too many sync wait commands"); split extras onto NoOps.
def _legalize_sem_waits(nc, mybir, max_waits=1):
    n = 0
    for f in nc.m.functions:
        for bb in f.blocks:
            new = []
            for inst in bb.instructions:
                si = inst.sync_info
                if si is not None and si.on_wait and len(si.on_wait) > max_waits:
                    waits = list(si.on_wait)
                    extra, keep = waits[:-max_waits], waits[-max_waits:]
                    k = 0
                    while extra:
                        chunk, extra = extra[:max_waits], extra[max_waits:]
                        new.append(
                            mybir.InstNoOp(
                                name=f"{inst.name}-ws{k}",
                                sync_info=mybir.SyncInfo(on_wait=chunk, on_update=[]),
                                bass_nofuse=True,
                                engine=inst.engine,
                            )
                        )
                        k += 1
                        n += 1
                    si.on_wait = keep
                new.append(inst)
            bb.instructions[:] = new
    return n


def _pack_pairs(rows):
    """rows: sorted 1-D int array of needed (rebased) slice rows.
    Returns (slots, sub) where slots[k] is the base row of pair-read k
    (covers rows slots[k], slots[k]+1) and sub[i] in {0,1} gives the
    sub-position of rows[i] inside its slot."""
    slots = []
    sub = np.zeros(len(rows), dtype=np.int64)
    i = 0
    n = len(rows)
    while i < n:
        v = rows[i]
        slots.append(v)
        sub[i] = 0
        if i + 1 < n and rows[i + 1] == v + 1:
            sub[i + 1] = 1
            i += 2
        else:
            i += 1
    return np.asarray(slots, dtype=np.int64), sub


def _pack_gaps(rows, maxgap):
    """Greedy pair-cover of sorted rows where a pair-read from shingle table
    gap-g covers rows (v, v+g), g in 1..maxgap.  Singles use the gap-1 table
    with a zero-count junk partner.  Returns per-class (1..maxgap) lists of
    (base_row, row_index_a, row_index_b_or_-1)."""
    classes = {g: [] for g in range(1, maxgap + 1)}
    i, n = 0, len(rows)
    while i < n:
        v = rows[i]
        if i + 1 < n and rows[i + 1] - v <= maxgap:
            g = int(rows[i + 1] - v)
            classes[g].append((v, i, i + 1))
            i += 2
        else:
            classes[1].append((v, i, -1))
            i += 1
    return classes


def _marshal_v2(tokens, lengths, emb_table, W, pairs=True, ks=0):
    """ks > 0: additionally stage, per core, the FIRST ks*128 rows of its
    vocab slice as a contiguous stream block (consumed by plain HWDGE DMA +
    DVE dot products, bypassing Q7 descriptor generation); only rows beyond
    the stream block are pair-gathered."""
    import ml_dtypes

    F8 = ml_dtypes.float8_e4m3
    tokens = np.asarray(tokens)
    lengths = np.asarray(lengths).astype(np.int64)
    emb_table = np.ascontiguousarray(emb_table, dtype=np.float32)

    mask = np.arange(L)[None, :] < lengths[:, None]
    flat_tok = tokens[mask].astype(np.int64)
    flat_b = np.broadcast_to(np.arange(B)[:, None], (B, L))[mask]
    uniq, inv = np.unique(flat_tok, return_inverse=True)
    U = len(uniq)
    cnt = np.zeros((U, B), dtype=np.float32)
    np.add.at(cnt, (inv, flat_b), 1.0)
    assert cnt.max() <= 16, "counts must be exact in fp8 e4m3"

    # shard by equalizing DESCRIPTOR count: greedy-cover globally with
    # gap<=maxgap pair reads, cut the desc list evenly, then each core
    # re-packs its rows (a cut can split a pair; costs <=1 desc per core).
    maxgap = int(os.environ.get("BERT_MAXGAP", "3"))
    gcls = _pack_gaps(uniq.astype(np.int64), maxgap)
    gdescs = sorted((ia for lst in gcls.values() for (_, ia, _) in lst))
    NSd = len(gdescs)
    row_bounds = []
    for c in range(NCORES + 1):
        k = NSd * c // NCORES
        row_bounds.append(gdescs[k] if k < NSd else U)
    # class emission order: small classes first, class 1 (with singles) last
    cls_order = [g for g in range(2, maxgap + 1)] + [1]

    per_core = []
    ncls_max = {g: 0 for g in cls_order}
    span_max = 0
    for c in range(NCORES):
        s, e = row_bounds[c], row_bounds[c + 1]
        lo = int(uniq[s])
        hi = int(uniq[e - 1]) + 1
        span = hi - lo
        assert span <= 32766, f"core {c} slice span {span} exceeds int16 gather range"
        rows = (uniq[s:e] - lo).astype(np.int64)
        scnt = cnt[s:e]  # aligned with rows
        classes = _pack_gaps(rows, maxgap)
        per_core.append((lo, span, rows, scnt, classes))
        for g in cls_order:
            ncls_max[g] = max(ncls_max[g], len(classes[g]))
        span_max = max(span_max, span)
    Ts = {g: max(1, -(-ncls_max[g] // 128)) for g in cls_order}
    Tsum = sum(Ts.values())

    wrep = np.zeros((128, PDIMP), dtype=np.float32)
    wrep[:, :PDIM] = np.asarray(W, dtype=np.float32).reshape(1, PDIM)

    in_maps = []
    for c in range(NCORES):
        lo, span, rows, scnt, classes = per_core[c]
        sl = np.zeros((span + maxgap, PDIMP), dtype=F8)
        sl[:span, :PDIM] = emb_table[lo : lo + span].astype(F8)

        im = {"wrep": wrep}
        idx_all = []
        sel_all = []
        for g in cls_order:
            # shingle table gap-g: embg[v] = (slice[v], slice[v+g])
            embg = np.zeros((span_max, PAIRE), dtype=F8)
            embg[:span, :PDIMP] = sl[:span]
            embg[:span, PDIMP:] = sl[g : span + g]
            im[f"embg{g}"] = embg

            Tg = Ts[g]
            lst = classes[g]
            idx = np.zeros(Tg * 128, dtype=np.int16)
            selm = np.zeros((Tg * 128, 2, B), dtype=F8)
            for j, (v, ia, ib) in enumerate(lst):
                idx[j] = v
                selm[j, 0] = scnt[ia].astype(F8)
                if ib >= 0:
                    selm[j, 1] = scnt[ib].astype(F8)
            idx_all.append(idx)
            sel_all.append(
                selm.reshape(Tg, 128, 2 * B).transpose(1, 0, 2).reshape(128, Tg * 2 * B)
            )
        idx = np.concatenate(idx_all)  # [Tsum*128]
        # wrapped layout: index i -> [i % 16, i // 16], replicated x8 bands
        wrapped = idx.reshape(Tsum * 8, 16).T  # [16, Tsum*8]
        im["idx16"] = np.tile(wrapped, (8, 1)).copy()  # [128, Tsum*8]
        im["sel"] = np.concatenate(sel_all, axis=1).copy()
        in_maps.append(im)
    return (Ts, cls_order), span_max, in_maps


def _chunk_sched(T, chunk):
    """Descending chunk sizes: big steady-state calls, small trailing calls so
    the last call's DMA drain + consume tail is short."""
    sched = []
    left = T
    while left > 14:
        sched.append(chunk)
        left -= chunk
    while left > 0:
        c = 4 if left > 4 else (left if left <= 2 else left - 2)
        c = min(c, left)
        sched.append(c)
        left -= c
    return sched


def _hoist_lib_load(nc, bass_isa, mybir):
    """Move the Pool ucode library reload into the framework preamble block so
    the ~9-12us Q7 boot overlaps the preamble/input loads instead of stalling
    the first gather.  Placed right after block 0's tile-guard memsets so the
    cross-engine tile-init handshake (which gates the input DMAs) is not
    pushed behind the boot drain."""
    f = nc.m.functions[0]
    reload_inst = None
    for bb in f.blocks:
        for k, inst in enumerate(bb.instructions):
            if isinstance(inst, bass_isa.InstPseudoReloadLibraryIndex):
                reload_inst = bb.instructions.pop(k)
                break
        if reload_inst is not None:
            break
    if reload_inst is not None:
        b0 = f.blocks[0].instructions
        pos = 0
        for k, inst in enumerate(b0):
            if isinstance(inst, mybir.InstMemset):
                pos = k + 1
        f.blocks[0].instructions.insert(pos, reload_inst)
    return reload_inst is not None


def _build_v2(Vmax, Tinfo, chunk, gbufs, dummyg=False, legalize=True, hoist=True, ks=0, wide=False):
    from concourse import bass, bacc, mybir, bass_isa
    import concourse.tile as tile

    F32 = mybir.dt.float32
    F8 = mybir.dt.float8e4
    I16 = mybir.dt.int16
    DR = mybir.MatmulPerfMode.DoubleRow

    Ts, cls_order = Tinfo
    T = sum(Ts.values())

    nc = bacc.Bacc(None, num_devices=NCORES)
    embgs = {
        g: nc.declare_dram_parameter(f"embg{g}", [Vmax, PAIRE], F8, isOutput=False)
        for g in cls_order
    }
    idx16 = nc.declare_dram_parameter("idx16", [128, T * 8], I16, isOutput=False)
    sel = nc.declare_dram_parameter("sel", [128, T * 2 * B], F8, isOutput=False)
    wrep = nc.declare_dram_parameter("wrep", [128, PDIMP], F32, isOutput=False)
    # [1, B] free-major: the final store is ONE contiguous 256B descriptor
    # (a [B,1] partition-major store is 64 tiny packets whose completion
    # semaphore lags ~5us)
    outp = nc.declare_dram_parameter("out", [1, B], F32, isOutput=True)

    # per-class call schedules: full calls for the leading small classes,
    # descending small-tail calls for the final (largest) class
    seg = []
    for gi, g in enumerate(cls_order):
        Tg = Ts[g]
        if gi == len(cls_order) - 1:
            sch = _chunk_sched(Tg, chunk)
        else:
            sch = [chunk] * (Tg // chunk) + ([Tg % chunk] if Tg % chunk else [])
        if gi == 0 and sch and sch[0] > 4:
            # split the global first call: the Tensor engine (which trails the
            # gather stream by ~9us) gets its first data ~7us sooner
            sch = [4, sch[0] - 4] + sch[1:]
        for c in sch:
            seg.append((g, c))
    sched = [c for _, c in seg]
    # phase split: accumulate everything up to the last call in PSUM pair 0
    # and reduce it against W while the last call still gathers/lands.
    t_phase = T - sched[-1] if len(sched) > 1 else T

    with tile.TileContext(nc) as tc:
        with (
            tc.tile_pool(name="meta", bufs=1) as meta,
            tc.tile_pool(name="g", bufs=gbufs) as gp,
            tc.tile_pool(name="ps", bufs=1, space="PSUM") as pp,
        ):
            # idx for the first call loads separately (tiny) so the first
            # gather starts as soon as the Q7 ucode is booted.
            c0 = sched[0]
            idxA_sb = meta.tile([128, c0 * 8], I16)
            nc.sync.dma_start(out=idxA_sb[:], in_=idx16[:, : c0 * 8])
            w_sb = meta.tile([128, PDIMP], F32)
            nc.sync.dma_start(out=w_sb[:], in_=wrep[:])
            sel_sb = meta.tile([128, T * 2 * B], F8)
            nc.sync.dma_start(out=sel_sb[:], in_=sel[:])
            idxB_sb = meta.tile([128, (T - c0) * 8], I16)
            nc.sync.dma_start(out=idxB_sb[:], in_=idx16[:, c0 * 8 :])

            if wide:
                pools = [
                    pp.tile([B, PDIMP], F32, tag="pw0", name="pw0"),
                    pp.tile([B, PDIMP], F32, tag="pw1", name="pw1"),
                ]
                scr_w = meta.tile([B, PDIMP], F32)
            else:
                pools = [
                    (
                        pp.tile([B, HALF], F32, tag="pa0", name="pa0"),
                        pp.tile([B, HALF], F32, tag="pb0", name="pb0"),
                    ),
                    (
                        pp.tile([B, HALF], F32, tag="pa1", name="pa1"),
                        pp.tile([B, HALF], F32, tag="pb1", name="pb1"),
                    ),
                ]
                scr_a = meta.tile([B, HALF], F32)
                scr_b = meta.tile([B, HALF], F32)
            ys = [meta.tile([B, 1], F32, name=f"y{i}") for i in range(4)]

            sel4 = sel_sb[:].rearrange("p (t two b) -> p t two b", two=2, b=B)

            def reduce_phase(ph, ya, yb):
                if wide:
                    nc.vector.scalar_tensor_tensor(
                        out=scr_w[:],
                        in0=pools[ph][:],
                        scalar=1.0,
                        in1=w_sb[:B, :],
                        op0=mybir.AluOpType.mult,
                        op1=mybir.AluOpType.mult,
                        accum_out=ya[:],
                    )
                    nc.vector.memset(yb[:], 0.0)
                    return
                pa, pb = pools[ph]
                nc.vector.scalar_tensor_tensor(
                    out=scr_a[:],
                    in0=pa[:],
                    scalar=1.0,
                    in1=w_sb[:B, 0:HALF],
                    op0=mybir.AluOpType.mult,
                    op1=mybir.AluOpType.mult,
                    accum_out=ya[:],
                )
                nc.vector.scalar_tensor_tensor(
                    out=scr_b[:],
                    in0=pb[:],
                    scalar=1.0,
                    in1=w_sb[:B, HALF:PDIMP],
                    op0=mybir.AluOpType.mult,
                    op1=mybir.AluOpType.mult,
                    accum_out=yb[:],
                )

            s = 0
            for gcls, c in seg:
                g = gp.tile([128, c, PAIRE], F8, tag="g")
                if s == 0:
                    idxs_ap = idxA_sb[:]
                else:
                    idxs_ap = idxB_sb[:, (s - c0) * 8 : (s - c0 + c) * 8]
                nc.gpsimd.dma_gather(
                    out_ap=g[:],
                    in_ap=embgs[gcls][:],
                    idxs_ap=idxs_ap,
                    num_idxs=c * 128,
                    num_idxs_reg=c * 128,
                    elem_size=PAIRE,
                )
                g4 = g[:].rearrange("p c (two h) -> p c two h", two=2)
                for j in range(c):
                    tt = s + j
                    ph = 0 if tt < t_phase else 1
                    lo_t = 0 if ph == 0 else t_phase
                    hi_t = (t_phase - 1) if ph == 0 else (T - 1)
                    lhsT = sel4[:, tt]
                    if wide:
                        nc.tensor.matmul(
                            out=pools[ph][:],
                            lhsT=lhsT,
                            rhs=g4[:, j],
                            start=(tt == lo_t),
                            stop=(tt == hi_t),
                            perf_mode=DR,
                        )
                    else:
                        pa, pb = pools[ph]
                        nc.tensor.matmul(
                            out=pa[:],
                            lhsT=lhsT,
                            rhs=g4[:, j, :, 0:HALF],
                            start=(tt == lo_t),
                            stop=(tt == hi_t),
                            perf_mode=DR,
                        )
                        nc.tensor.matmul(
                            out=pb[:],
                            lhsT=lhsT,
                            rhs=g4[:, j, :, HALF:PDIMP],
                            start=(tt == lo_t),
                            stop=(tt == hi_t),
                            perf_mode=DR,
                        )
                    if tt == t_phase - 1 and t_phase < T:
                        reduce_phase(0, ys[0], ys[1])
                s += c

            # [B,B] identity for the final PE transpose (Pool is free after
            # the last gather; DVE converts int compare to f32 one-hots)
            iot = meta.tile([B, B], mybir.dt.int32)
            nc.gpsimd.iota(iot[:], pattern=[[1, B]], base=0, channel_multiplier=-1)
            idf = meta.tile([B, B], F32)
            nc.vector.tensor_scalar(
                out=idf[:],
                in0=iot[:],
                scalar1=0,
                scalar2=None,
                op0=mybir.AluOpType.is_equal,
            )

            y = meta.tile([B, 1], F32)
            if t_phase < T:
                reduce_phase(1, ys[2], ys[3])
                ysum = meta.tile([B, 1], F32)
                nc.vector.tensor_tensor(
                    out=ysum[:], in0=ys[0][:], in1=ys[1][:], op=mybir.AluOpType.add
                )
                nc.vector.tensor_tensor(
                    out=y[:], in0=ys[2][:], in1=ys[3][:], op=mybir.AluOpType.add
                )
                nc.vector.tensor_tensor(
                    out=y[:], in0=y[:], in1=ysum[:], op=mybir.AluOpType.add
                )
            else:
                reduce_phase(0, ys[0], ys[1])
                nc.vector.tensor_tensor(
                    out=y[:], in0=ys[0][:], in1=ys[1][:], op=mybir.AluOpType.add
                )
            # transpose [B,1] -> [1,B] so the store is one contiguous packet;
            # the stream partial (sum over z partitions) accumulates into the
            # same PSUM tile via a ones-matmul in the same accumulation group.
            yt_ps = pp.tile([1, B], F32, tag="yt")
            nc.tensor.transpose(out=yt_ps[:], in_=y[:], identity=idf[:])
            o_sb = meta.tile([1, B], F32)
            nc.vector.tensor_copy(out=o_sb[:], in_=yt_ps[:])
            nc.sync.dma_start(out=outp[:], in_=o_sb[:])

    nc.compile()
    if hoist:
        _hoist_lib_load(nc, bass_isa, mybir)
    if legalize:
        _legalize_sem_waits(nc, __import__("concourse.mybir", fromlist=["x"]))
    return nc


def _kernel_v2(tokens, lengths, emb_table, W, b):
    from concourse.bass_utils import run_bass_kernel_spmd

    chunk = int(os.environ.get("BERT_CHUNK", "8"))
    gbufs = int(os.environ.get("BERT_GBUFS", "4"))
    dummyg = os.environ.get("BERT_DUMMYG", "0") == "1"
    trace = os.environ.get("BERT_TRACE", "0") == "1"
    wide = os.environ.get("BERT_WIDE", "0") == "1"

    T, Vmax, in_maps = _marshal_v2(tokens, lengths, emb_table, W)
    nc = _build_v2(Vmax, T, chunk, gbufs, dummyg=dummyg, wide=wide)
    res = run_bass_kernel_spmd(nc, in_maps, core_ids=list(range(NCORES)), trace=trace)
    LAST["results"] = res
    LAST["T"] = T
    LAST["Vmax"] = Vmax
    total = np.zeros(B, dtype=np.float64)
    for c in range(NCORES):
        total += res.results[c]["out"].reshape(B).astype(np.float64)
    z = total / float(L) + float(np.asarray(b).reshape(-1)[0])
    out = 1.0 / (1.0 + np.exp(-z))
    return out.astype(np.float32)


def kernel(tokens, lengths, emb_table, W, b):
    if os.environ.get("BERT_V", "2") == "2":
        return _kernel_v2(tokens, lengths, emb_table, W, b)
    return _kernel_seq(tokens, lengths, emb_table, W, b)


# ---------------------------------------------------------------------------
# v1 (BERT_V=1): sequence-ownership variant — each core owns 8 length-balanced
# sequences end-to-end (table replicated in bf16, int16 gathers windowed into
# 32768-row vocab slabs, no collective).
WIN = 32768
NW = -(-VOCAB // WIN)
NSEQ = B // NCORES


def _marshal_seq(tokens, lengths, emb_table, W, b, dtype="bf16"):
    import ml_dtypes

    sdt = ml_dtypes.bfloat16 if dtype == "bf16" else np.float32
    tokens = np.asarray(tokens)
    lengths = np.asarray(lengths).astype(np.int64)

    # per-sequence unique-token histograms over vocab windows; greedy
    # vector-balancing assignment minimizes sum_w max_c rows (the padded
    # tile count is driven by per-window maxima, not total length)
    order = np.argsort(-lengths, kind="stable")
    hists = np.zeros((B, NW), dtype=np.int64)
    for bidx in range(B):
        u = np.unique(tokens[bidx, : lengths[bidx]].astype(np.int64))
        hists[bidx] = np.bincount(u // WIN, minlength=NW)
    Wc = np.zeros((NCORES, NW), dtype=np.int64)
    counts = np.zeros(NCORES, dtype=np.int64)
    assign = np.full((NCORES, NSEQ), -1, dtype=np.int64)
    for bidx in order:
        cands = np.where(counts < NSEQ)[0]
        best, bobj = None, None
        for c in cands:
            trial = Wc.copy()
            trial[c] += hists[bidx]
            obj = trial.max(axis=0).sum()
            if bobj is None or obj < bobj:
                best, bobj = c, obj
        assign[best, counts[best]] = bidx
        counts[best] += 1
        Wc[best] += hists[bidx]

    def _obj(Wm):
        return (-(-Wm.max(axis=0) // 128)).sum() * 1000000 + Wm.max(axis=0).sum()

    # swap refinement: directly minimize padded tile count sum_w ceil(max/128)
    for _ in range(40):
        improved = False
        cur = _obj(Wc)
        for c1 in range(NCORES):
            for j1 in range(NSEQ):
                for c2 in range(c1 + 1, NCORES):
                    for j2 in range(NSEQ):
                        b1, b2 = assign[c1, j1], assign[c2, j2]
                        trial = Wc.copy()
                        trial[c1] += hists[b2] - hists[b1]
                        trial[c2] += hists[b1] - hists[b2]
                        if _obj(trial) < cur:
                            assign[c1, j1], assign[c2, j2] = b2, b1
                            Wc = trial
                            cur = _obj(Wc)
                            improved = True
        if not improved:
            break

    per_core_rows = []  # (uniq, cnt8) per core
    for c in range(NCORES):
        toks = np.concatenate(
            [tokens[assign[c, j], : lengths[assign[c, j]]] for j in range(NSEQ)]
        ).astype(np.int64)
        locb = np.concatenate(
            [np.full(int(lengths[assign[c, j]]), j, dtype=np.int64) for j in range(NSEQ)]
        )
        uniq, inv = np.unique(toks, return_inverse=True)
        cnt8 = np.zeros((len(uniq), NSEQ), dtype=np.float32)
        np.add.at(cnt8, (inv, locb), 1.0)
        per_core_rows.append((uniq, cnt8))

    # per-window tile counts, common across cores (SPMD: same program)
    Tw = []
    bnds = []
    for w in range(NW):
        lo, hi = w * WIN, min((w + 1) * WIN, VOCAB)
        per_core_bnd = [
            (np.searchsorted(u, lo), np.searchsorted(u, hi)) for u, _ in per_core_rows
        ]
        bnds.append(per_core_bnd)
        Tw.append(max(-(-int(e - s) // 128) for s, e in per_core_bnd))
    T = sum(Tw)

    emb16 = np.zeros((VOCAB, PDIMP), dtype=sdt)
    emb16[:, :PDIM] = np.ascontiguousarray(emb_table, dtype=np.float32).astype(sdt)
    wdt = np.float32 if os.environ.get("BERT_SEQSPLIT", "1") == "1" else sdt
    wrep = np.broadcast_to(
        np.asarray(W, dtype=np.float32).astype(wdt).reshape(1, PDIM), (128, PDIM)
    ).copy()
    brep = np.full((NSEQ, 1), np.float32(np.asarray(b).reshape(-1)[0]), dtype=np.float32)

    in_maps = []
    for c in range(NCORES):
        uniq, cnt8 = per_core_rows[c]
        rows = np.zeros(T * 128, dtype=np.int16)
        selm = np.zeros((T * 128, NSEQ), dtype=np.float32)
        t0 = 0
        for w in range(NW):
            s0, e0 = bnds[w][c]
            n = int(e0 - s0)
            rows[t0 * 128 : t0 * 128 + n] = (uniq[s0:e0] - w * WIN).astype(np.int16)
            selm[t0 * 128 : t0 * 128 + n] = cnt8[s0:e0]
            t0 += Tw[w]
        if os.environ.get("BERT_SEQSPLIT", "1") == "1":
            selm = selm.astype(sdt)
        wrapped = rows.reshape(T * 8, 16).T  # [16, T*8]
        in_maps.append(
            {
                "emb": emb16,
                "idx16": np.tile(wrapped, (8, 1)).copy(),
                "sel": selm.reshape(T, 128, NSEQ)
                .transpose(1, 0, 2)
                .reshape(128, T * NSEQ)
                .copy(),
                "wrep": wrep,
                "brep": brep,
            }
        )
    return Tw, in_maps, assign


def _build_seq(Tw, chunk, gbufs, ybufs, dtype="bf16", legalize=True, split=True):
    from concourse import bacc, mybir
    import concourse.tile as tile

    F32 = mybir.dt.float32
    GDT = mybir.dt.bfloat16 if dtype == "bf16" else F32
    I16 = mybir.dt.int16
    T = sum(Tw)

    scratch = int(os.environ.get("BERT_DMASCRATCH", "131072"))
    nc = bacc.Bacc(None, num_devices=NCORES, dynamic_dma_scratch_size=scratch)
    emb = nc.declare_dram_parameter("emb", [VOCAB, PDIMP], GDT, isOutput=False)
    idx16 = nc.declare_dram_parameter("idx16", [128, T * 8], I16, isOutput=False)
    SELDT = GDT if split else F32
    sel = nc.declare_dram_parameter("sel", [128, T * NSEQ], SELDT, isOutput=False)
    WDT = F32 if split else GDT
    wrep = nc.declare_dram_parameter("wrep", [128, PDIM], WDT, isOutput=False)
    brep = nc.declare_dram_parameter("brep", [NSEQ, 1], F32, isOutput=False)
    outp = nc.declare_dram_parameter("out", [1, NSEQ], F32, isOutput=True)

    with tile.TileContext(nc) as tc:
        with (
            tc.tile_pool(name="meta", bufs=1) as meta,
            tc.tile_pool(name="g", bufs=gbufs) as gp,
            tc.tile_pool(name="y", bufs=ybufs) as yp,
            tc.tile_pool(name="ps", bufs=1, space="PSUM") as pp,
        ):
            idx16_sb = meta.tile([128, T * 8], I16)
            nc.sync.dma_start(out=idx16_sb[:], in_=idx16[:])
            sel_sb = meta.tile([128, T * NSEQ], SELDT)
            nc.sync.dma_start(out=sel_sb[:], in_=sel[:])
            w_sb = meta.tile([128, PDIM], WDT)
            nc.sync.dma_start(out=w_sb[:], in_=wrep[:])
            b_sb = meta.tile([NSEQ, 1], F32)
            nc.sync.dma_start(out=b_sb[:], in_=brep[:])

            dot_ps = pp.tile([1, NSEQ], F32)
            first_chunk = True
            HALFP = PDIM // 2
            if split:
                pe_set = set(range(1, T, 2))
                dot8 = pp.tile([NSEQ, 1], F32, tag="d8")
                pool_a = pp.tile([NSEQ, HALFP], F32, tag="pa")
                pool_b = pp.tile([NSEQ, HALFP], F32, tag="pb")
                w16 = meta.tile([128, PDIM], GDT)
                nc.vector.tensor_copy(out=w16[:], in_=w_sb[:])
            else:
                pe_set = set()
                w16 = w_sb
            stt_set = set(range(T)) - pe_set
            pe_lo, pe_hi = (min(pe_set), max(pe_set)) if pe_set else (0, 0)
            st_lo, st_hi = (min(stt_set), max(stt_set)) if stt_set else (0, 0)
            t = 0
            for w in range(NW):
                wlo = w * WIN
                whi = min(wlo + WIN, VOCAB)
                left = Tw[w]
                while left > 0:
                    c = min(4 if first_chunk else chunk, left)
                    first_chunk = False
                    g = gp.tile([128, c, PDIMP], GDT, tag="g")
                    nc.gpsimd.dma_gather(
                        out_ap=g[:],
                        in_ap=emb[wlo:whi],
                        idxs_ap=idx16_sb[:, t * 8 : (t + c) * 8],
                        num_idxs=c * 128,
                        num_idxs_reg=c * 128,
                        elem_size=PDIMP,
                    )
                    gflat = g[:].rearrange("p c e -> p (c e)")
                    for j in range(c):
                        tt = t + j
                        off = j * PDIMP
                        if tt in pe_set:
                            nc.tensor.matmul(
                                out=pool_a[:],
                                lhsT=sel_sb[:, tt * NSEQ : (tt + 1) * NSEQ],
                                rhs=gflat[:, off : off + HALFP],
                                start=(tt == pe_lo),
                                stop=(tt == pe_hi),
                            )
                            nc.tensor.matmul(
                                out=pool_b[:],
                                lhsT=sel_sb[:, tt * NSEQ : (tt + 1) * NSEQ],
                                rhs=gflat[:, off + HALFP : off + PDIM],
                                start=(tt == pe_lo),
                                stop=(tt == pe_hi),
                            )
                            continue
                        y = yp.tile([128, 1], GDT if split else F32)
                        gs = gflat[:, off : off + PDIM]
                        nc.vector.scalar_tensor_tensor(
                            out=gs,
                            in0=gs,
                            scalar=1.0,
                            in1=w16[:],
                            op0=mybir.AluOpType.mult,
                            op1=mybir.AluOpType.mult,
                            accum_out=y[:],
                        )
                        if split:
                            nc.tensor.matmul(
                                out=dot8[:],
                                lhsT=sel_sb[:, tt * NSEQ : (tt + 1) * NSEQ],
                                rhs=y[:],
                                start=(tt == st_lo),
                                stop=(tt == st_hi),
                            )
                        else:
                            nc.tensor.matmul(
                                out=dot_ps[:],
                                lhsT=y[:],
                                rhs=sel_sb[:, tt * NSEQ : (tt + 1) * NSEQ],
                                start=(tt == st_lo),
                                stop=(tt == st_hi),
                            )
                    t += c
                    left -= c

            if split:
                pooled_sb = meta.tile([NSEQ, PDIM], F32)
                nc.vector.tensor_copy(out=pooled_sb[:, :HALFP], in_=pool_a[:])
                nc.vector.tensor_copy(out=pooled_sb[:, HALFP:], in_=pool_b[:])
                scr = meta.tile([NSEQ, PDIM], F32)
                y8 = meta.tile([NSEQ, 1], F32)
                nc.vector.scalar_tensor_tensor(
                    out=scr[:],
                    in0=pooled_sb[:],
                    scalar=1.0,
                    in1=w_sb[:NSEQ, :],
                    op0=mybir.AluOpType.mult,
                    op1=mybir.AluOpType.mult,
                    accum_out=y8[:],
                )
                part = meta.tile([NSEQ, 1], F32)
                nc.vector.tensor_tensor(
                    out=part[:], in0=dot8[:], in1=y8[:], op=mybir.AluOpType.add
                )
                o_sb = meta.tile([NSEQ, 1], F32)
                nc.scalar.activation(
                    out=o_sb[:],
                    in_=part[:],
                    func=mybir.ActivationFunctionType.Sigmoid,
                    bias=b_sb[:],
                    scale=1.0 / float(L),
                )
                nc.sync.dma_start(out=outp[0, :, None], in_=o_sb[:])
            else:
                o_sb = meta.tile([1, NSEQ], F32)
                nc.scalar.activation(
                    out=o_sb[:],
                    in_=dot_ps[:],
                    func=mybir.ActivationFunctionType.Sigmoid,
                    bias=b_sb[:1, :],
                    scale=1.0 / float(L),
                )
                nc.sync.dma_start(out=outp[:], in_=o_sb[:])

    nc.compile()
    if legalize:
        _legalize_sem_waits(nc, __import__("concourse.mybir", fromlist=["x"]))
    return nc


def _kernel_seq(tokens, lengths, emb_table, W, b):
    from concourse.bass_utils import run_bass_kernel_spmd

    dtype = os.environ.get("BERT_DTYPE", "bf16")
    chunk = int(os.environ.get("BERT_CHUNK", "8"))
    gbufs = int(os.environ.get("BERT_GBUFS", "4"))
    ybufs = int(os.environ.get("BERT_YBUFS", "16"))
    trace = os.environ.get("BERT_TRACE", "0") == "1"

    split = os.environ.get("BERT_SEQSPLIT", "1") == "1"
    Tw, in_maps, assign = _marshal_seq(tokens, lengths, emb_table, W, b, dtype=dtype)
    nc = _build_seq(Tw, chunk, gbufs, ybufs, dtype=dtype, split=split)
    res = run_bass_kernel_spmd(nc, in_maps, core_ids=list(range(NCORES)), trace=trace)
    LAST["results"] = res
    LAST["T"] = sum(Tw)
    LAST["Vmax"] = VOCAB
    out = np.zeros(B, dtype=np.float32)
    for c in range(NCORES):
        vals = res.results[c]["out"].reshape(-1)
        for j in range(NSEQ):
            out[assign[c, j]] = vals[j]
    return out
